# revision 42
# baseline (speedup 1.0000x reference)
"""DownscaleLabel Trainium2 kernel.

Input:  label [8, 1024, 1024] int32, values in [-1, 6] (-1 = ignore).
Output: [8, 1, 64, 64] int32. Per 16x16 block: the dominant real class c
        (0..6) if its pixel count >= 192 (= 0.75 * 256), else -1.

Key simplification: since 192 > 128, at most one class can reach the
threshold, so no argmax / tie-breaking is needed:
    out = -1 + sum_c (c+1) * [count_c >= 192]

Per-core algorithm (one 1024x1024 image per NeuronCore, batch-sharded):
  1. For each of 8 row-tiles [128, 1024], two clampless "packed one-hot"
     encodings, each a single affine producing the int16 bf16 bit pattern
     (bitcast to bf16 afterwards):
       lo (ACT Copy affine): 2^(15-5x) -> classes 2,1,0 in 5-bit fields
         1..3; x=3 / x=-1 land in junk fields 0 / 4.
       hi (DVE mult+add):    2^(5x-10) -> classes 3..6 in fields 1..4;
         x<=2 maps to field 0 / fractions 2^-5..2^-15 whose per-block sum
         stays < 1 (absorbed by truncation + junk field 0).
  2. PE matmul against a block-diagonal ones matrix pools 16 rows exactly
     into PSUM fp32 (all sums <= 2^24, exact); tile t's pattern places its
     8 block-rows at the right partitions, accumulating across tiles.
  3. Cast PSUM to int32 (ACT relu / DVE copy), extract field pairs
     ((1,3) and (2,4), 10 bits apart so 256-max block sums cannot carry),
     col-pool 16 with tensor_reduce, split, threshold at count >= 192,
     weight by (class id + 1) per partition, PE fold-matmul sums the two
     partition halves, subtract 1, DMA out [64, 64] int32.

All class counts are exact; at most one class can reach 192, so the
weighted-mask sum needs no argmax or tie-breaking.
"""

import sys

import numpy as np

_BASS_REPO = "/opt/trn_rl_repo"

H = W = 1024
SC = 16
TH = TW = 64
P = 128
NT = H // P  # 8 row-tiles
N_CORES = 8


def _ensure_path():
    if _BASS_REPO not in sys.path:
        sys.path.insert(0, _BASS_REPO)


def make_consts():
    """Host-side constant tensors fed as kernel inputs."""
    import ml_dtypes

    # poolw[k, 72*t + k//16] = 1 for t in 0..7: eight [128, 64] block-diagonal
    # row-pooling patterns (pattern t places tile t's 8 block-rows at out
    # partitions 8t + k//16 within a 64-partition psum group).  Columns
    # 512:576 hold the final fold pattern (k, k % 64) summing the two
    # partition halves of the acc tile via the PE.
    poolw = np.zeros((P, 576), dtype=np.float32)
    k = np.arange(P)
    for t in range(NT):
        poolw[k, 72 * t + k // 16] = 1.0
    poolw[k, 512 + (k % 64)] = 1.0
    poolw = poolw.astype(ml_dtypes.bfloat16)

    # wcol[p, f]: output weight (class id + 1) for field f on partition p.
    # lo plane (partitions 0-63), enc = 2^(15-5x): f0=junk(3), f1=c2,
    # f2=c1, f3=c0, f4=junk(-1).  hi plane (64-127), enc = 2^(5x-10):
    # f0=junk(<=2), f1=c3, f2=c4, f3=c5, f4=c6.  Field 0 on both planes
    # absorbs the bounded fraction round-up, so only fields 1-4 matter.
    wcol = np.zeros((P, 8), dtype=np.float32)
    wcol[:64, 1] = 3.0
    wcol[:64, 2] = 2.0
    wcol[:64, 3] = 1.0
    wcol[64:, 1] = 4.0
    wcol[64:, 2] = 5.0
    wcol[64:, 3] = 6.0
    wcol[64:, 4] = 7.0
    return poolw, wcol


def emit_downscale(ctx, tc, out_ap, label_ap, poolw_ap, wcol_ap):
    """Emit the per-core kernel body into TileContext tc."""
    _ensure_path()
    from concourse import mybir
    from concourse.alu_op_type import AluOpType as aop

    nc = tc.nc
    dt = mybir.dt

    cpool = ctx.enter_context(tc.tile_pool(name="consts", bufs=1))
    xpool = ctx.enter_context(tc.tile_pool(name="x", bufs=NT))
    epool = ctx.enter_context(tc.tile_pool(name="e", bufs=4))
    ppool = ctx.enter_context(tc.tile_pool(name="psum", bufs=1, space="PSUM"))
    spool = ctx.enter_context(tc.tile_pool(name="small", bufs=1))
    fpool = ctx.enter_context(tc.tile_pool(name="f", bufs=2))

    # Tiles 0 and 1 load individually so encoding starts as early as
    # possible, then the tiny consts (pw gates the matmuls, which start
    # later), then the remaining six tiles as three 1 MiB transfers.
    items = []
    for t in range(2):
        xt = xpool.tile([P, W], dt.int32, tag="xs")
        nc.sync.dma_start(xt[:], label_ap[P * t : P * (t + 1), :])
        items.append((t, xt[:], 0))

    pw = cpool.tile([P, 576], dt.bfloat16)
    nc.sync.dma_start(pw[:], poolw_ap)
    bz = cpool.tile([P, 1], dt.float32)
    nc.vector.memset(bz[:], 0.0)

    for g in range(1, NT // 2):
        xg = xpool.tile([P, 2 * W], dt.int32, tag="x")
        nc.sync.dma_start(
            xg[:].rearrange("p (t c) -> p t c", t=2),
            label_ap[2 * P * g : 2 * P * (g + 1), :].rearrange("(t p) c -> p t c", p=P),
        )
        items.append((2 * g, xg[:, 0:W], 0))
        items.append((2 * g + 1, xg[:, W : 2 * W], 0))

    # wc only gates the final thresholding; load it after the input stream.
    wc = cpool.tile([P, 8], dt.float32)
    nc.sync.dma_start(wc[:], wcol_ap)

    # Packed row-pooled counts: partitions 0-63 = lo plane (block-rows in
    # order), 64-127 = hi plane.  Fields are 5 bits at bits 0,5,10,15,20.
    # Planes live in separate PSUM bank pairs (cols 0:1024 / 1024:2048)
    # because accumulation-group zeroing is bank-wide across partitions.
    psum = ppool.tile([P, 2 * W], dt.float32)

    for t, x, coff in items:
        width = x.shape[-1]

        # Clampless encodings: the int16 result is the bf16 bit pattern of
        # 2^(15-5x) (lo, ACT Copy affine) / 2^(5x-10) (hi, DVE).  Out-of-
        # plane classes become fractions 2^-5..2^-20 whose per-block total
        # stays < 1; field 0 absorbs the bounded round-up.
        el = epool.tile([P, W], dt.int16, tag="el")
        nc.scalar.activation(
            el[:, 0:width], x, mybir.ActivationFunctionType.Copy,
            bias=18176.0, scale=-640.0,
        )
        eh = epool.tile([P, W], dt.int16, tag="eh")
        nc.vector.tensor_scalar(eh[:, 0:width], x, 640, 14976, aop.mult, aop.add)

        for plane, e in ((0, el), (1, eh)):
            base = 64 * plane
            rhs = e[:].bitcast(dt.bfloat16)
            for h in range(width // 512):
                col = coff + 512 * h
                nc.tensor.matmul(
                    psum[base : base + 64, W * plane + col : W * plane + col + 512],
                    pw[:, 64 * t : 64 * (t + 1)],
                    rhs[:, 512 * h : 512 * (h + 1)],
                    start=(t == 0),
                    stop=(t == NT - 1),
                )

    # Downstream: unpack fields, col-pool, threshold, combine.  The casts
    # run on ACT (Relu == identity for the non-negative packed sums), which
    # is idle once the encodings finish; fp32 -> int32 converts on output.
    vi = spool.tile([P, W], dt.int32, tag="vi")
    nc.scalar.activation(
        vi[0:64, :],
        psum[0:64, 0:W],
        mybir.ActivationFunctionType.Relu,
        bias=bz[0:64, :],
        scale=1.0,
    )
    nc.vector.tensor_copy(vi[64:128, :], psum[64:128, W : 2 * W])

    # Paired field extraction: fields (0,2) and (1,3) extracted together 10
    # bits apart (block sums <= 256 < 1024, so no cross-field carry), field
    # 4 alone.  Then col-pool 16, split pairs, threshold, weight.
    PAIRMASK = 31 | (31 << 10)
    red = {}
    for j, (shift, mask) in enumerate(((5, PAIRMASK), (10, PAIRMASK))):
        fk = fpool.tile([P, W], dt.int32, tag="fk")
        nc.vector.tensor_scalar(
            fk[:], vi[:], shift, mask, aop.logical_shift_right, aop.bitwise_and
        )
        r = spool.tile([P, TW], dt.int32, tag=f"red{j}")
        # int32 accumulation of small exact ints.
        with nc.allow_low_precision(reason="small int counts, exact"):
            nc.vector.tensor_reduce(
                r[:],
                fk[:].rearrange("p (b s) -> p b s", s=SC),
                mybir.AxisListType.X,
                aop.add,
            )
        red[j] = r

    # Per-field block counts: pair j holds fields j+1 (low 10 bits) and
    # j+3 (high bits).
    cnts = {}
    for k in range(1, 5):
        j = (k - 1) % 2
        c = spool.tile([P, TW], dt.int32, tag=f"cnt{k}")
        if k < 3:
            nc.vector.tensor_scalar(c[:], red[j][:], 1023, None, aop.bitwise_and)
        else:
            nc.vector.tensor_scalar(c[:], red[j][:], 10, None, aop.logical_shift_right)
        cnts[k] = c

    acc = None
    for k in range(1, 5):
        mk = spool.tile([P, TW], dt.int32, tag=f"mk{k}")
        nc.vector.tensor_scalar(mk[:], cnts[k][:], 192, None, aop.is_ge)
        if acc is None:
            acc = spool.tile([P, TW], dt.bfloat16, tag=f"acc{k}")
            nc.vector.tensor_scalar(acc[:], mk[:], wc[:, k : k + 1], None, aop.mult)
        else:
            acc2 = spool.tile([P, TW], dt.bfloat16, tag=f"acc{k}")
            nc.vector.scalar_tensor_tensor(
                acc2[:], mk[:], wc[:, k : k + 1], acc[:], aop.mult, aop.add
            )
            acc = acc2

    # Cross-partition combine on the PE: fold pattern sums partitions p and
    # p+64 into psum row p (values <= 7, exact in bf16).
    psum_f = ppool.tile([TH, TW], dt.float32, tag="psum_f")
    nc.tensor.matmul(psum_f[:], pw[:, 512:576], acc[:], start=True, stop=True)
    resi = spool.tile([TH, TW], dt.int32, tag="resi")
    nc.vector.tensor_scalar(resi[:], psum_f[:], 1, None, aop.subtract)
    nc.sync.dma_start(out_ap, resi[:])


def _split_multi_waits(nc):
    """This toolchain's walrus codegen accepts at most ONE semaphore wait per
    engine instruction (two on EventSemaphore).  The Tile scheduler sometimes
    emits more; spill the extras onto same-engine NoOp carriers inserted just
    before the instruction (engines dispatch in order, so the carrier's wait
    is satisfied before the instruction issues -- semantics preserved)."""
    _ensure_path()
    from concourse import mybir

    for func in nc.m.functions:
        for blk in func.blocks:
            insts = blk.instructions
            out = []
            changed = False
            for ins in insts:
                si = ins.sync_info
                cap = 2 if isinstance(ins, mybir.InstEventSemaphore) else 1
                if si and si.on_wait and len(si.on_wait) > cap:
                    waits = list(si.on_wait)
                    for w in waits[:-cap]:
                        out.append(
                            mybir.InstNoOp(
                                name=nc.get_next_instruction_name(),
                                engine=ins.engine,
                                sync_info=mybir.SyncInfo(on_wait=[w], on_update=[]),
                                bass_nofuse=True,
                            )
                        )
                    si.on_wait = waits[-cap:]
                    changed = True
                out.append(ins)
            if changed:
                blk.instructions = out


def _install_ntff_hook():
    """Provide antenv.axon_hooks + the ctypes NTFF profile hook when the
    agent image lacks them (mirrors trn_agent_boot.trn_boot section 6)."""
    import contextlib
    import ctypes
    import types

    try:
        from antenv.axon_hooks import get_axon_ntff_profile_hook  # noqa: F401

        return
    except ImportError:
        pass
    _ensure_path()
    import antenv

    so_path = "/opt/axon/libaxon_pjrt.so"
    try:
        lib = ctypes.CDLL(so_path)
    except OSError:
        return
    if not hasattr(lib, "axon_start_nrt_profile"):
        return
    lib.axon_start_nrt_profile.argtypes = [
        ctypes.POINTER(ctypes.c_int64),
        ctypes.c_size_t,
    ]
    lib.axon_start_nrt_profile.restype = ctypes.c_int64
    lib.axon_stop_nrt_profile.argtypes = [ctypes.c_char_p]
    lib.axon_stop_nrt_profile.restype = ctypes.c_int64

    @contextlib.contextmanager
    def _hook(output_dir, device_ids):
        import jax

        jax.devices()
        if device_ids:
            ids = (ctypes.c_int64 * len(device_ids))(*device_ids)
            rc = lib.axon_start_nrt_profile(ids, len(device_ids))
        else:
            rc = lib.axon_start_nrt_profile(None, 0)
        if rc != 0:
            raise RuntimeError(f"axon_start_nrt_profile rc={rc}")
        try:
            yield
        finally:
            n = lib.axon_stop_nrt_profile(str(output_dir).encode())
            print(f"ntff profile: {n} file(s) written to {output_dir}", file=sys.stderr)

    mod = types.ModuleType("antenv.axon_hooks")
    _h = [_hook]
    mod.set_axon_ntff_profile_hook = lambda h: _h.__setitem__(0, h)
    mod.get_axon_ntff_profile_hook = lambda: _h[0]
    sys.modules["antenv.axon_hooks"] = mod
    antenv.axon_hooks = mod

    # upload_artifacts pushes the NEFF dir to a cloud bucket; keep local.
    from concourse import bass_utils as _bu

    _bu.upload_artifacts = lambda tmpdir: tmpdir


_NC_CACHE = None


def _build_nc():
    global _NC_CACHE
    if _NC_CACHE is not None:
        return _NC_CACHE
    _ensure_path()
    from contextlib import ExitStack

    import concourse.bass as bass
    import concourse.tile as tile
    from concourse import mybir

    dt = mybir.dt
    nc = bass.Bass("TRN2", target_bir_lowering=False, debug=False)
    label = nc.dram_tensor("label", [H, W], dt.int32, kind="ExternalInput").ap()
    poolw = nc.dram_tensor("poolw", [P, 576], dt.bfloat16, kind="ExternalInput").ap()
    wcol = nc.dram_tensor("wcol", [P, 8], dt.float32, kind="ExternalInput").ap()
    out = nc.dram_tensor("out", [TH, TW], dt.int32, kind="ExternalOutput").ap()
    with tile.TileContext(nc) as tc:
        with ExitStack() as ctx:
            emit_downscale(ctx, tc, out, label, poolw, wcol)
    _split_multi_waits(nc)
    _NC_CACHE = nc
    return nc


def run_on_hw(label, trace=False):
    """Run on the 8 NeuronCores; returns (out [8,1,64,64] int32, exec_time_ns)."""
    _ensure_path()
    from concourse.bass_utils import run_bass_kernel_spmd

    if trace:
        _install_ntff_hook()
    nc = _build_nc()
    poolw, wcol = make_consts()
    label = np.ascontiguousarray(label, dtype=np.int32)
    in_maps = [
        {"label": label[i], "poolw": poolw, "wcol": wcol} for i in range(N_CORES)
    ]
    r = run_bass_kernel_spmd(nc, in_maps, core_ids=list(range(N_CORES)), trace=trace)
    outs = np.stack([r.results[i]["out"] for i in range(N_CORES)])
    return outs.reshape(8, 1, TH, TW).astype(np.int32), r.exec_time_ns


def kernel(label):
    out, _ = run_on_hw(label, trace=False)
    return out


# revision 43
# speedup vs baseline: 1.0375x; 1.0375x over previous
"""DownscaleLabel Trainium2 kernel.

Input:  label [8, 1024, 1024] int32, values in [-1, 6] (-1 = ignore).
Output: [8, 1, 64, 64] int32. Per 16x16 block: the dominant real class c
        (0..6) if its pixel count >= 192 (= 0.75 * 256), else -1.

Key simplification: since 192 > 128, at most one class can reach the
threshold, so no argmax / tie-breaking is needed:
    out = -1 + sum_c (c+1) * [count_c >= 192]

Per-core algorithm (one 1024x1024 image per NeuronCore, batch-sharded):
  1. For each of 8 row-tiles [128, 1024], two clampless "packed one-hot"
     encodings, each a single affine producing the int16 bf16 bit pattern
     (bitcast to bf16 afterwards):
       lo (ACT Copy affine): 2^(15-5x) -> classes 2,1,0 in 5-bit fields
         1..3; x=3 / x=-1 land in junk fields 0 / 4.
       hi (DVE mult+add):    2^(5x-10) -> classes 3..6 in fields 1..4;
         x<=2 maps to field 0 / fractions 2^-5..2^-15 whose per-block sum
         stays < 1 (absorbed by truncation + junk field 0).
  2. PE matmul against a block-diagonal ones matrix pools 16 rows exactly
     into PSUM fp32 (all sums <= 2^24, exact); tile t's pattern places its
     8 block-rows at the right partitions, accumulating across tiles.
  3. Cast PSUM to int32 (ACT relu / DVE copy), extract field pairs
     ((1,3) and (2,4), 10 bits apart so 256-max block sums cannot carry),
     col-pool 16 with tensor_reduce, split, threshold at count >= 192,
     weight by (class id + 1) per partition, PE fold-matmul sums the two
     partition halves, subtract 1, DMA out [64, 64] int32.

All class counts are exact; at most one class can reach 192, so the
weighted-mask sum needs no argmax or tie-breaking.
"""

import sys

import numpy as np

_BASS_REPO = "/opt/trn_rl_repo"

H = W = 1024
SC = 16
TH = TW = 64
P = 128
NT = H // P  # 8 row-tiles
N_CORES = 8


def _ensure_path():
    if _BASS_REPO not in sys.path:
        sys.path.insert(0, _BASS_REPO)


def make_consts():
    """Host-side constant tensors fed as kernel inputs."""
    import ml_dtypes

    # poolw[k, 72*t + k//16] = 1 for t in 0..7: eight [128, 64] block-diagonal
    # row-pooling patterns (pattern t places tile t's 8 block-rows at out
    # partitions 8t + k//16 within a 64-partition psum group).  Columns
    # 512:576 hold the final fold pattern (k, k % 64) summing the two
    # partition halves of the acc tile via the PE.
    poolw = np.zeros((P, 576), dtype=np.float32)
    k = np.arange(P)
    for t in range(NT):
        poolw[k, 72 * t + k // 16] = 1.0
    poolw[k, 512 + (k % 64)] = 1.0
    poolw = poolw.astype(ml_dtypes.bfloat16)

    # wcol[p, f]: output weight (class id + 1) for field f on partition p.
    # lo plane (partitions 0-63), enc = 2^(15-5x): f0=junk(3), f1=c2,
    # f2=c1, f3=c0, f4=junk(-1).  hi plane (64-127), enc = 2^(5x-10):
    # f0=junk(<=2), f1=c3, f2=c4, f3=c5, f4=c6.  Field 0 on both planes
    # absorbs the bounded fraction round-up, so only fields 1-4 matter.
    wcol = np.zeros((P, 8), dtype=np.float32)
    wcol[:64, 1] = 3.0
    wcol[:64, 2] = 2.0
    wcol[:64, 3] = 1.0
    wcol[64:, 1] = 4.0
    wcol[64:, 2] = 5.0
    wcol[64:, 3] = 6.0
    wcol[64:, 4] = 7.0
    return poolw, wcol


def emit_downscale(ctx, tc, out_ap, label_ap, poolw_ap, wcol_ap):
    """Emit the per-core kernel body into TileContext tc."""
    _ensure_path()
    from concourse import mybir
    from concourse.alu_op_type import AluOpType as aop

    nc = tc.nc
    dt = mybir.dt

    cpool = ctx.enter_context(tc.tile_pool(name="consts", bufs=1))
    xpool = ctx.enter_context(tc.tile_pool(name="x", bufs=NT))
    epool = ctx.enter_context(tc.tile_pool(name="e", bufs=4))
    ppool = ctx.enter_context(tc.tile_pool(name="psum", bufs=1, space="PSUM"))
    spool = ctx.enter_context(tc.tile_pool(name="small", bufs=1))
    fpool = ctx.enter_context(tc.tile_pool(name="f", bufs=2))

    # Tiles 0 and 1 load individually so encoding starts as early as
    # possible, then the tiny consts (pw gates the matmuls, which start
    # later), then the remaining six tiles as three 1 MiB transfers.
    items = []
    for t in range(2):
        xt = xpool.tile([P, W], dt.int32, tag="xs")
        nc.sync.dma_start(xt[:], label_ap[P * t : P * (t + 1), :])
        items.append((t, xt[:], 0))

    pw = cpool.tile([P, 576], dt.bfloat16)
    nc.sync.dma_start(pw[:], poolw_ap)
    wc = cpool.tile([P, 8], dt.float32)
    nc.sync.dma_start(wc[:], wcol_ap)
    bz = cpool.tile([P, 1], dt.float32)
    nc.vector.memset(bz[:], 0.0)

    for g in range(1, NT // 2):
        xg = xpool.tile([P, 2 * W], dt.int32, tag="x")
        nc.sync.dma_start(
            xg[:].rearrange("p (t c) -> p t c", t=2),
            label_ap[2 * P * g : 2 * P * (g + 1), :].rearrange("(t p) c -> p t c", p=P),
        )
        items.append((2 * g, xg[:, 0:W], 0))
        items.append((2 * g + 1, xg[:, W : 2 * W], 0))

    # Packed row-pooled counts: partitions 0-63 = lo plane (block-rows in
    # order), 64-127 = hi plane.  Fields are 5 bits at bits 0,5,10,15,20.
    # Planes live in separate PSUM bank pairs (cols 0:1024 / 1024:2048)
    # because accumulation-group zeroing is bank-wide across partitions.
    psum = ppool.tile([P, 2 * W], dt.float32)

    for t, x, coff in items:
        width = x.shape[-1]

        # Clampless encodings: the int16 result is the bf16 bit pattern of
        # 2^(15-5x) (lo, ACT Copy affine) / 2^(5x-10) (hi, DVE).  Out-of-
        # plane classes become fractions 2^-5..2^-20 whose per-block total
        # stays < 1; field 0 absorbs the bounded round-up.
        el = epool.tile([P, W], dt.int16, tag="el")
        nc.scalar.activation(
            el[:, 0:width], x, mybir.ActivationFunctionType.Copy,
            bias=18176.0, scale=-640.0,
        )
        eh = epool.tile([P, W], dt.int16, tag="eh")
        nc.vector.tensor_scalar(eh[:, 0:width], x, 640, 14976, aop.mult, aop.add)

        for plane, e in ((0, el), (1, eh)):
            base = 64 * plane
            rhs = e[:].bitcast(dt.bfloat16)
            for h in range(width // 512):
                col = coff + 512 * h
                nc.tensor.matmul(
                    psum[base : base + 64, W * plane + col : W * plane + col + 512],
                    pw[:, 64 * t : 64 * (t + 1)],
                    rhs[:, 512 * h : 512 * (h + 1)],
                    start=(t == 0),
                    stop=(t == NT - 1),
                )

    # Downstream: unpack fields, col-pool, threshold, combine.  The casts
    # run on ACT (Relu == identity for the non-negative packed sums), which
    # is idle once the encodings finish; fp32 -> int32 converts on output.
    vi = spool.tile([P, W], dt.int32, tag="vi")
    nc.scalar.activation(
        vi[0:64, :],
        psum[0:64, 0:W],
        mybir.ActivationFunctionType.Relu,
        bias=bz[0:64, :],
        scale=1.0,
    )
    nc.vector.tensor_copy(vi[64:128, :], psum[64:128, W : 2 * W])

    # Paired field extraction: fields (0,2) and (1,3) extracted together 10
    # bits apart (block sums <= 256 < 1024, so no cross-field carry), field
    # 4 alone.  Then col-pool 16, split pairs, threshold, weight.
    PAIRMASK = 31 | (31 << 10)
    red = {}
    for j, (shift, mask) in enumerate(((5, PAIRMASK), (10, PAIRMASK))):
        fk = fpool.tile([P, W], dt.int32, tag="fk")
        nc.vector.tensor_scalar(
            fk[:], vi[:], shift, mask, aop.logical_shift_right, aop.bitwise_and
        )
        r = spool.tile([P, TW], dt.int32, tag=f"red{j}")
        # int32 accumulation of small exact ints.
        with nc.allow_low_precision(reason="small int counts, exact"):
            nc.vector.tensor_reduce(
                r[:],
                fk[:].rearrange("p (b s) -> p b s", s=SC),
                mybir.AxisListType.X,
                aop.add,
            )
        red[j] = r

    # Per-field block counts: pair j holds fields j+1 (low 10 bits) and
    # j+3 (high bits).
    cnts = {}
    for k in range(1, 5):
        j = (k - 1) % 2
        c = spool.tile([P, TW], dt.int32, tag=f"cnt{k}")
        if k < 3:
            nc.vector.tensor_scalar(c[:], red[j][:], 1023, None, aop.bitwise_and)
        else:
            nc.vector.tensor_scalar(c[:], red[j][:], 10, None, aop.logical_shift_right)
        cnts[k] = c

    acc = None
    for k in range(1, 5):
        mk = spool.tile([P, TW], dt.int32, tag=f"mk{k}")
        nc.vector.tensor_scalar(mk[:], cnts[k][:], 192, None, aop.is_ge)
        if acc is None:
            acc = spool.tile([P, TW], dt.bfloat16, tag=f"acc{k}")
            nc.vector.tensor_scalar(acc[:], mk[:], wc[:, k : k + 1], None, aop.mult)
        else:
            acc2 = spool.tile([P, TW], dt.bfloat16, tag=f"acc{k}")
            nc.vector.scalar_tensor_tensor(
                acc2[:], mk[:], wc[:, k : k + 1], acc[:], aop.mult, aop.add
            )
            acc = acc2

    # Cross-partition combine on the PE: fold pattern sums partitions p and
    # p+64 into psum row p (values <= 7, exact in bf16).
    psum_f = ppool.tile([TH, TW], dt.float32, tag="psum_f")
    nc.tensor.matmul(psum_f[:], pw[:, 512:576], acc[:], start=True, stop=True)
    resi = spool.tile([TH, TW], dt.int32, tag="resi")
    nc.vector.tensor_scalar(resi[:], psum_f[:], 1, None, aop.subtract)
    nc.sync.dma_start(out_ap, resi[:])


def _split_multi_waits(nc):
    """This toolchain's walrus codegen accepts at most ONE semaphore wait per
    engine instruction (two on EventSemaphore).  The Tile scheduler sometimes
    emits more; spill the extras onto same-engine NoOp carriers inserted just
    before the instruction (engines dispatch in order, so the carrier's wait
    is satisfied before the instruction issues -- semantics preserved)."""
    _ensure_path()
    from concourse import mybir

    for func in nc.m.functions:
        for blk in func.blocks:
            insts = blk.instructions
            out = []
            changed = False
            for ins in insts:
                si = ins.sync_info
                cap = 2 if isinstance(ins, mybir.InstEventSemaphore) else 1
                if si and si.on_wait and len(si.on_wait) > cap:
                    waits = list(si.on_wait)
                    for w in waits[:-cap]:
                        out.append(
                            mybir.InstNoOp(
                                name=nc.get_next_instruction_name(),
                                engine=ins.engine,
                                sync_info=mybir.SyncInfo(on_wait=[w], on_update=[]),
                                bass_nofuse=True,
                            )
                        )
                    si.on_wait = waits[-cap:]
                    changed = True
                out.append(ins)
            if changed:
                blk.instructions = out


def _install_ntff_hook():
    """Provide antenv.axon_hooks + the ctypes NTFF profile hook when the
    agent image lacks them (mirrors trn_agent_boot.trn_boot section 6)."""
    import contextlib
    import ctypes
    import types

    try:
        from antenv.axon_hooks import get_axon_ntff_profile_hook  # noqa: F401

        return
    except ImportError:
        pass
    _ensure_path()
    import antenv

    so_path = "/opt/axon/libaxon_pjrt.so"
    try:
        lib = ctypes.CDLL(so_path)
    except OSError:
        return
    if not hasattr(lib, "axon_start_nrt_profile"):
        return
    lib.axon_start_nrt_profile.argtypes = [
        ctypes.POINTER(ctypes.c_int64),
        ctypes.c_size_t,
    ]
    lib.axon_start_nrt_profile.restype = ctypes.c_int64
    lib.axon_stop_nrt_profile.argtypes = [ctypes.c_char_p]
    lib.axon_stop_nrt_profile.restype = ctypes.c_int64

    @contextlib.contextmanager
    def _hook(output_dir, device_ids):
        import jax

        jax.devices()
        if device_ids:
            ids = (ctypes.c_int64 * len(device_ids))(*device_ids)
            rc = lib.axon_start_nrt_profile(ids, len(device_ids))
        else:
            rc = lib.axon_start_nrt_profile(None, 0)
        if rc != 0:
            raise RuntimeError(f"axon_start_nrt_profile rc={rc}")
        try:
            yield
        finally:
            n = lib.axon_stop_nrt_profile(str(output_dir).encode())
            print(f"ntff profile: {n} file(s) written to {output_dir}", file=sys.stderr)

    mod = types.ModuleType("antenv.axon_hooks")
    _h = [_hook]
    mod.set_axon_ntff_profile_hook = lambda h: _h.__setitem__(0, h)
    mod.get_axon_ntff_profile_hook = lambda: _h[0]
    sys.modules["antenv.axon_hooks"] = mod
    antenv.axon_hooks = mod

    # upload_artifacts pushes the NEFF dir to a cloud bucket; keep local.
    from concourse import bass_utils as _bu

    _bu.upload_artifacts = lambda tmpdir: tmpdir


_NC_CACHE = None


def _build_nc():
    global _NC_CACHE
    if _NC_CACHE is not None:
        return _NC_CACHE
    _ensure_path()
    from contextlib import ExitStack

    import concourse.bass as bass
    import concourse.tile as tile
    from concourse import mybir

    dt = mybir.dt
    nc = bass.Bass("TRN2", target_bir_lowering=False, debug=False)
    label = nc.dram_tensor("label", [H, W], dt.int32, kind="ExternalInput").ap()
    poolw = nc.dram_tensor("poolw", [P, 576], dt.bfloat16, kind="ExternalInput").ap()
    wcol = nc.dram_tensor("wcol", [P, 8], dt.float32, kind="ExternalInput").ap()
    out = nc.dram_tensor("out", [TH, TW], dt.int32, kind="ExternalOutput").ap()
    with tile.TileContext(nc) as tc:
        with ExitStack() as ctx:
            emit_downscale(ctx, tc, out, label, poolw, wcol)
    _split_multi_waits(nc)
    _NC_CACHE = nc
    return nc


def run_on_hw(label, trace=False):
    """Run on the 8 NeuronCores; returns (out [8,1,64,64] int32, exec_time_ns)."""
    _ensure_path()
    from concourse.bass_utils import run_bass_kernel_spmd

    if trace:
        _install_ntff_hook()
    nc = _build_nc()
    poolw, wcol = make_consts()
    label = np.ascontiguousarray(label, dtype=np.int32)
    in_maps = [
        {"label": label[i], "poolw": poolw, "wcol": wcol} for i in range(N_CORES)
    ]
    r = run_bass_kernel_spmd(nc, in_maps, core_ids=list(range(N_CORES)), trace=trace)
    outs = np.stack([r.results[i]["out"] for i in range(N_CORES)])
    return outs.reshape(8, 1, TH, TW).astype(np.int32), r.exec_time_ns


def kernel(label):
    out, _ = run_on_hw(label, trace=False)
    return out


# revision 45
# speedup vs baseline: 1.0578x; 1.0195x over previous
"""DownscaleLabel Trainium2 kernel.

Input:  label [8, 1024, 1024] int32, values in [-1, 6] (-1 = ignore).
Output: [8, 1, 64, 64] int32. Per 16x16 block: the dominant real class c
        (0..6) if its pixel count >= 192 (= 0.75 * 256), else -1.

Key simplification: since 192 > 128, at most one class can reach the
threshold, so no argmax / tie-breaking is needed:
    out = -1 + sum_c (c+1) * [count_c >= 192]

Per-core algorithm (one 1024x1024 image per NeuronCore, batch-sharded):
  1. For each of 8 row-tiles [128, 1024], two clampless "packed one-hot"
     encodings, each a single affine producing the int16 bf16 bit pattern
     (bitcast to bf16 afterwards):
       lo (ACT Copy affine): 2^(15-5x) -> classes 2,1,0 in 5-bit fields
         1..3; x=3 / x=-1 land in junk fields 0 / 4.
       hi (DVE mult+add):    2^(5x-10) -> classes 3..6 in fields 1..4;
         x<=2 maps to field 0 / fractions 2^-5..2^-15 whose per-block sum
         stays < 1 (absorbed by truncation + junk field 0).
  2. PE matmul against a block-diagonal ones matrix pools 16 rows exactly
     into PSUM fp32 (all sums <= 2^24, exact); tile t's pattern places its
     8 block-rows at the right partitions, accumulating across tiles.
  3. Cast PSUM to int32 (ACT relu / DVE copy), extract field pairs
     ((1,3) and (2,4), 10 bits apart so 256-max block sums cannot carry),
     col-pool 16 with tensor_reduce, split, threshold at count >= 192,
     weight by (class id + 1) per partition, PE fold-matmul sums the two
     partition halves, subtract 1, DMA out [64, 64] int32.

All class counts are exact; at most one class can reach 192, so the
weighted-mask sum needs no argmax or tie-breaking.
"""

import sys

import numpy as np

_BASS_REPO = "/opt/trn_rl_repo"

H = W = 1024
SC = 16
TH = TW = 64
P = 128
NT = H // P  # 8 row-tiles
N_CORES = 8


def _ensure_path():
    if _BASS_REPO not in sys.path:
        sys.path.insert(0, _BASS_REPO)


def make_consts():
    """Host-side constant tensors fed as kernel inputs."""
    import ml_dtypes

    # poolw[k, 72*t + k//16] = 1 for t in 0..7: eight [128, 64] block-diagonal
    # row-pooling patterns (pattern t places tile t's 8 block-rows at out
    # partitions 8t + k//16 within a 64-partition psum group).  Columns
    # 512:576 hold the final fold pattern (k, k % 64) summing the two
    # partition halves of the acc tile via the PE.
    poolw = np.zeros((P, 576), dtype=np.float32)
    k = np.arange(P)
    for t in range(NT):
        poolw[k, 72 * t + k // 16] = 1.0
    poolw[k, 512 + (k % 64)] = 1.0
    poolw = poolw.astype(ml_dtypes.bfloat16)

    # wcol[p, f]: output weight (class id + 1) for field f on partition p.
    # lo plane (partitions 0-63), enc = 2^(15-5x): f0=junk(3), f1=c2,
    # f2=c1, f3=c0, f4=junk(-1).  hi plane (64-127), enc = 2^(5x-10):
    # f0=junk(<=2), f1=c3, f2=c4, f3=c5, f4=c6.  Field 0 on both planes
    # absorbs the bounded fraction round-up, so only fields 1-4 matter.
    wcol = np.zeros((P, 8), dtype=np.float32)
    wcol[:64, 1] = 3.0
    wcol[:64, 2] = 2.0
    wcol[:64, 3] = 1.0
    wcol[64:, 1] = 4.0
    wcol[64:, 2] = 5.0
    wcol[64:, 3] = 6.0
    wcol[64:, 4] = 7.0
    return poolw, wcol


def emit_downscale(ctx, tc, out_ap, label_ap, poolw_ap, wcol_ap):
    """Emit the per-core kernel body into TileContext tc."""
    _ensure_path()
    from concourse import mybir
    from concourse.alu_op_type import AluOpType as aop

    nc = tc.nc
    dt = mybir.dt

    cpool = ctx.enter_context(tc.tile_pool(name="consts", bufs=1))
    xpool = ctx.enter_context(tc.tile_pool(name="x", bufs=NT))
    # Keeping this (otherwise-unused) pool reproduces the SBUF layout of the
    # best-measured schedule (36.4us); removing it measured ~2.5us slower.
    ctx.enter_context(tc.tile_pool(name="t", bufs=4))
    epool = ctx.enter_context(tc.tile_pool(name="e", bufs=4))
    ppool = ctx.enter_context(tc.tile_pool(name="psum", bufs=1, space="PSUM"))
    spool = ctx.enter_context(tc.tile_pool(name="small", bufs=1))
    fpool = ctx.enter_context(tc.tile_pool(name="f", bufs=2))

    # Tiles 0 and 1 load individually so encoding starts as early as
    # possible, then the tiny consts (pw gates the matmuls, which start
    # later), then the remaining six tiles as three 1 MiB transfers.
    items = []
    for t in range(2):
        xt = xpool.tile([P, W], dt.int32, tag="xs")
        nc.sync.dma_start(xt[:], label_ap[P * t : P * (t + 1), :])
        items.append((t, xt[:], 0))

    pw = cpool.tile([P, 576], dt.bfloat16)
    nc.sync.dma_start(pw[:], poolw_ap)
    wc = cpool.tile([P, 8], dt.float32)
    nc.sync.dma_start(wc[:], wcol_ap)
    bz = cpool.tile([P, 1], dt.float32)
    nc.vector.memset(bz[:], 0.0)

    for g in range(1, NT // 2):
        xg = xpool.tile([P, 2 * W], dt.int32, tag="x")
        nc.sync.dma_start(
            xg[:].rearrange("p (t c) -> p t c", t=2),
            label_ap[2 * P * g : 2 * P * (g + 1), :].rearrange("(t p) c -> p t c", p=P),
        )
        items.append((2 * g, xg[:, 0:W], 0))
        items.append((2 * g + 1, xg[:, W : 2 * W], 0))

    # Packed row-pooled counts: partitions 0-63 = lo plane (block-rows in
    # order), 64-127 = hi plane.  Fields are 5 bits at bits 0,5,10,15,20.
    # Planes live in separate PSUM bank pairs (cols 0:1024 / 1024:2048)
    # because accumulation-group zeroing is bank-wide across partitions.
    psum = ppool.tile([P, 2 * W], dt.float32)

    for t, x, coff in items:
        width = x.shape[-1]

        # Clampless encodings: the int16 result is the bf16 bit pattern of
        # 2^(15-5x) (lo, ACT Copy affine) / 2^(5x-10) (hi, DVE).  Out-of-
        # plane classes become fractions 2^-5..2^-20 whose per-block total
        # stays < 1; field 0 absorbs the bounded round-up.
        el = epool.tile([P, W], dt.int16, tag="el")
        nc.scalar.activation(
            el[:, 0:width], x, mybir.ActivationFunctionType.Copy,
            bias=18176.0, scale=-640.0,
        )
        eh = epool.tile([P, W], dt.int16, tag="eh")
        nc.vector.tensor_scalar(eh[:, 0:width], x, 640, 14976, aop.mult, aop.add)

        for plane, e in ((0, el), (1, eh)):
            base = 64 * plane
            rhs = e[:].bitcast(dt.bfloat16)
            for h in range(width // 512):
                col = coff + 512 * h
                nc.tensor.matmul(
                    psum[base : base + 64, W * plane + col : W * plane + col + 512],
                    pw[:, 64 * t : 64 * (t + 1)],
                    rhs[:, 512 * h : 512 * (h + 1)],
                    start=(t == 0),
                    stop=(t == NT - 1),
                )

    # Downstream: unpack fields, col-pool, threshold, combine.  The casts
    # run on ACT (Relu == identity for the non-negative packed sums), which
    # is idle once the encodings finish; fp32 -> int32 converts on output.
    vi = spool.tile([P, W], dt.int32, tag="vi")
    nc.scalar.activation(
        vi[0:64, :],
        psum[0:64, 0:W],
        mybir.ActivationFunctionType.Relu,
        bias=bz[0:64, :],
        scale=1.0,
    )
    nc.vector.tensor_copy(vi[64:128, :], psum[64:128, W : 2 * W])

    # Paired field extraction: fields (0,2) and (1,3) extracted together 10
    # bits apart (block sums <= 256 < 1024, so no cross-field carry), field
    # 4 alone.  Then col-pool 16, split pairs, threshold, weight.
    PAIRMASK = 31 | (31 << 10)
    red = {}
    for j, (shift, mask) in enumerate(((5, PAIRMASK), (10, PAIRMASK))):
        fk = fpool.tile([P, W], dt.int32, tag="fk")
        nc.vector.tensor_scalar(
            fk[:], vi[:], shift, mask, aop.logical_shift_right, aop.bitwise_and
        )
        r = spool.tile([P, TW], dt.int32, tag=f"red{j}")
        # int32 accumulation of small exact ints.
        with nc.allow_low_precision(reason="small int counts, exact"):
            nc.vector.tensor_reduce(
                r[:],
                fk[:].rearrange("p (b s) -> p b s", s=SC),
                mybir.AxisListType.X,
                aop.add,
            )
        red[j] = r

    # Per-field block counts: pair j holds fields j+1 (low 10 bits) and
    # j+3 (high bits).
    cnts = {}
    for k in range(1, 5):
        j = (k - 1) % 2
        c = spool.tile([P, TW], dt.int32, tag=f"cnt{k}")
        if k < 3:
            nc.vector.tensor_scalar(c[:], red[j][:], 1023, None, aop.bitwise_and)
        else:
            nc.vector.tensor_scalar(c[:], red[j][:], 10, None, aop.logical_shift_right)
        cnts[k] = c

    acc = None
    for k in range(1, 5):
        mk = spool.tile([P, TW], dt.int32, tag=f"mk{k}")
        nc.vector.tensor_scalar(mk[:], cnts[k][:], 192, None, aop.is_ge)
        if acc is None:
            acc = spool.tile([P, TW], dt.bfloat16, tag=f"acc{k}")
            nc.vector.tensor_scalar(acc[:], mk[:], wc[:, k : k + 1], None, aop.mult)
        else:
            acc2 = spool.tile([P, TW], dt.bfloat16, tag=f"acc{k}")
            nc.vector.scalar_tensor_tensor(
                acc2[:], mk[:], wc[:, k : k + 1], acc[:], aop.mult, aop.add
            )
            acc = acc2

    # Cross-partition combine on the PE: fold pattern sums partitions p and
    # p+64 into psum row p (values <= 7, exact in bf16).
    psum_f = ppool.tile([TH, TW], dt.float32, tag="psum_f")
    nc.tensor.matmul(psum_f[:], pw[:, 512:576], acc[:], start=True, stop=True)
    resi = spool.tile([TH, TW], dt.int32, tag="resi")
    nc.vector.tensor_scalar(resi[:], psum_f[:], 1, None, aop.subtract)
    nc.sync.dma_start(out_ap, resi[:])


def _split_multi_waits(nc):
    """This toolchain's walrus codegen accepts at most ONE semaphore wait per
    engine instruction (two on EventSemaphore).  The Tile scheduler sometimes
    emits more; spill the extras onto same-engine NoOp carriers inserted just
    before the instruction (engines dispatch in order, so the carrier's wait
    is satisfied before the instruction issues -- semantics preserved)."""
    _ensure_path()
    from concourse import mybir

    for func in nc.m.functions:
        for blk in func.blocks:
            insts = blk.instructions
            out = []
            changed = False
            for ins in insts:
                si = ins.sync_info
                cap = 2 if isinstance(ins, mybir.InstEventSemaphore) else 1
                if si and si.on_wait and len(si.on_wait) > cap:
                    waits = list(si.on_wait)
                    for w in waits[:-cap]:
                        out.append(
                            mybir.InstNoOp(
                                name=nc.get_next_instruction_name(),
                                engine=ins.engine,
                                sync_info=mybir.SyncInfo(on_wait=[w], on_update=[]),
                                bass_nofuse=True,
                            )
                        )
                    si.on_wait = waits[-cap:]
                    changed = True
                out.append(ins)
            if changed:
                blk.instructions = out


def _install_ntff_hook():
    """Provide antenv.axon_hooks + the ctypes NTFF profile hook when the
    agent image lacks them (mirrors trn_agent_boot.trn_boot section 6)."""
    import contextlib
    import ctypes
    import types

    try:
        from antenv.axon_hooks import get_axon_ntff_profile_hook  # noqa: F401

        return
    except ImportError:
        pass
    _ensure_path()
    import antenv

    so_path = "/opt/axon/libaxon_pjrt.so"
    try:
        lib = ctypes.CDLL(so_path)
    except OSError:
        return
    if not hasattr(lib, "axon_start_nrt_profile"):
        return
    lib.axon_start_nrt_profile.argtypes = [
        ctypes.POINTER(ctypes.c_int64),
        ctypes.c_size_t,
    ]
    lib.axon_start_nrt_profile.restype = ctypes.c_int64
    lib.axon_stop_nrt_profile.argtypes = [ctypes.c_char_p]
    lib.axon_stop_nrt_profile.restype = ctypes.c_int64

    @contextlib.contextmanager
    def _hook(output_dir, device_ids):
        import jax

        jax.devices()
        if device_ids:
            ids = (ctypes.c_int64 * len(device_ids))(*device_ids)
            rc = lib.axon_start_nrt_profile(ids, len(device_ids))
        else:
            rc = lib.axon_start_nrt_profile(None, 0)
        if rc != 0:
            raise RuntimeError(f"axon_start_nrt_profile rc={rc}")
        try:
            yield
        finally:
            n = lib.axon_stop_nrt_profile(str(output_dir).encode())
            print(f"ntff profile: {n} file(s) written to {output_dir}", file=sys.stderr)

    mod = types.ModuleType("antenv.axon_hooks")
    _h = [_hook]
    mod.set_axon_ntff_profile_hook = lambda h: _h.__setitem__(0, h)
    mod.get_axon_ntff_profile_hook = lambda: _h[0]
    sys.modules["antenv.axon_hooks"] = mod
    antenv.axon_hooks = mod

    # upload_artifacts pushes the NEFF dir to a cloud bucket; keep local.
    from concourse import bass_utils as _bu

    _bu.upload_artifacts = lambda tmpdir: tmpdir


_NC_CACHE = None


def _build_nc():
    global _NC_CACHE
    if _NC_CACHE is not None:
        return _NC_CACHE
    _ensure_path()
    from contextlib import ExitStack

    import concourse.bass as bass
    import concourse.tile as tile
    from concourse import mybir

    dt = mybir.dt
    nc = bass.Bass("TRN2", target_bir_lowering=False, debug=False)
    label = nc.dram_tensor("label", [H, W], dt.int32, kind="ExternalInput").ap()
    poolw = nc.dram_tensor("poolw", [P, 576], dt.bfloat16, kind="ExternalInput").ap()
    wcol = nc.dram_tensor("wcol", [P, 8], dt.float32, kind="ExternalInput").ap()
    out = nc.dram_tensor("out", [TH, TW], dt.int32, kind="ExternalOutput").ap()
    with tile.TileContext(nc) as tc:
        with ExitStack() as ctx:
            emit_downscale(ctx, tc, out, label, poolw, wcol)
    _split_multi_waits(nc)
    _NC_CACHE = nc
    return nc


def run_on_hw(label, trace=False):
    """Run on the 8 NeuronCores; returns (out [8,1,64,64] int32, exec_time_ns)."""
    _ensure_path()
    from concourse.bass_utils import run_bass_kernel_spmd

    if trace:
        _install_ntff_hook()
    nc = _build_nc()
    poolw, wcol = make_consts()
    label = np.ascontiguousarray(label, dtype=np.int32)
    in_maps = [
        {"label": label[i], "poolw": poolw, "wcol": wcol} for i in range(N_CORES)
    ]
    r = run_bass_kernel_spmd(nc, in_maps, core_ids=list(range(N_CORES)), trace=trace)
    outs = np.stack([r.results[i]["out"] for i in range(N_CORES)])
    return outs.reshape(8, 1, TH, TW).astype(np.int32), r.exec_time_ns


def kernel(label):
    out, _ = run_on_hw(label, trace=False)
    return out


# revision 46
# speedup vs baseline: 1.1204x; 1.0592x over previous
"""DownscaleLabel Trainium2 kernel.

Input:  label [8, 1024, 1024] int32, values in [-1, 6] (-1 = ignore).
Output: [8, 1, 64, 64] int32. Per 16x16 block: the dominant real class c
        (0..6) if its pixel count >= 192 (= 0.75 * 256), else -1.

Key simplification: since 192 > 128, at most one class can reach the
threshold, so no argmax / tie-breaking is needed:
    out = -1 + sum_c (c+1) * [count_c >= 192]

Per-core algorithm (one 1024x1024 image per NeuronCore, batch-sharded):
  1. For each of 8 row-tiles [128, 1024], two clampless "packed one-hot"
     encodings, each a single affine producing the int16 bf16 bit pattern
     (bitcast to bf16 afterwards):
       lo (ACT Copy affine): 2^(15-5x) -> classes 2,1,0 in 5-bit fields
         1..3; x=3 / x=-1 land in junk fields 0 / 4.
       hi (DVE mult+add):    2^(5x-10) -> classes 3..6 in fields 1..4;
         x<=2 maps to field 0 / fractions 2^-5..2^-15 whose per-block sum
         stays < 1 (absorbed by truncation + junk field 0).
  2. PE matmul against a block-diagonal ones matrix pools 16 rows exactly
     into PSUM fp32 (all sums <= 2^24, exact); tile t's pattern places its
     8 block-rows at the right partitions, accumulating across tiles.
  3. Cast PSUM to int32 (ACT relu / DVE copy), extract field pairs
     ((1,3) and (2,4), 10 bits apart so 256-max block sums cannot carry),
     col-pool 16 with tensor_reduce, split, threshold at count >= 192,
     weight by (class id + 1) per partition, PE fold-matmul sums the two
     partition halves, subtract 1, DMA out [64, 64] int32.

All class counts are exact; at most one class can reach 192, so the
weighted-mask sum needs no argmax or tie-breaking.
"""

import sys

import numpy as np

_BASS_REPO = "/opt/trn_rl_repo"

H = W = 1024
SC = 16
TH = TW = 64
P = 128
NT = H // P  # 8 row-tiles
N_CORES = 8


def _ensure_path():
    if _BASS_REPO not in sys.path:
        sys.path.insert(0, _BASS_REPO)


def make_consts():
    """Host-side constant tensors fed as kernel inputs."""
    import ml_dtypes

    # poolw[k, 72*t + k//16] = 1 for t in 0..7: eight [128, 64] block-diagonal
    # row-pooling patterns (pattern t places tile t's 8 block-rows at out
    # partitions 8t + k//16 within a 64-partition psum group).  Columns
    # 512:576 hold the final fold pattern (k, k % 64) summing the two
    # partition halves of the acc tile via the PE.
    poolw = np.zeros((P, 576), dtype=np.float32)
    k = np.arange(P)
    for t in range(NT):
        poolw[k, 72 * t + k // 16] = 1.0
    poolw[k, 512 + (k % 64)] = 1.0
    poolw = poolw.astype(ml_dtypes.bfloat16)

    # wcol[p, f]: output weight (class id + 1) for field f on partition p.
    # lo plane (partitions 0-63), enc = 2^(15-5x): f0=junk(3), f1=c2,
    # f2=c1, f3=c0, f4=junk(-1).  hi plane (64-127), enc = 2^(5x-10):
    # f0=junk(<=2), f1=c3, f2=c4, f3=c5, f4=c6.  Field 0 on both planes
    # absorbs the bounded fraction round-up, so only fields 1-4 matter.
    wcol = np.zeros((P, 8), dtype=np.float32)
    wcol[:64, 1] = 3.0
    wcol[:64, 2] = 2.0
    wcol[:64, 3] = 1.0
    wcol[64:, 1] = 4.0
    wcol[64:, 2] = 5.0
    wcol[64:, 3] = 6.0
    wcol[64:, 4] = 7.0
    return poolw, wcol


def emit_downscale(ctx, tc, out_ap, label_ap, poolw_ap, wcol_ap):
    """Emit the per-core kernel body into TileContext tc."""
    _ensure_path()
    from concourse import mybir
    from concourse.alu_op_type import AluOpType as aop

    nc = tc.nc
    dt = mybir.dt

    cpool = ctx.enter_context(tc.tile_pool(name="consts", bufs=1))
    xpool = ctx.enter_context(tc.tile_pool(name="x", bufs=NT))
    # Keeping this (otherwise-unused) pool reproduces the SBUF layout of the
    # best-measured schedule (36.4us); removing it measured ~2.5us slower.
    ctx.enter_context(tc.tile_pool(name="t", bufs=4))
    epool = ctx.enter_context(tc.tile_pool(name="e", bufs=4))
    ppool = ctx.enter_context(tc.tile_pool(name="psum", bufs=1, space="PSUM"))
    spool = ctx.enter_context(tc.tile_pool(name="small", bufs=1))
    fpool = ctx.enter_context(tc.tile_pool(name="f", bufs=2))

    # Tiles 0 and 1 load individually so encoding starts as early as
    # possible, then the tiny consts (pw gates the matmuls, which start
    # later), then the remaining six tiles as three 1 MiB transfers.
    items = []
    for t in range(2):
        xt = xpool.tile([P, W], dt.int32, tag="xs")
        nc.sync.dma_start(xt[:], label_ap[P * t : P * (t + 1), :])
        items.append((t, xt[:], 0))

    pw = cpool.tile([P, 576], dt.bfloat16)
    nc.sync.dma_start(pw[:], poolw_ap)
    wc = cpool.tile([P, 8], dt.float32)
    nc.sync.dma_start(wc[:], wcol_ap)
    bz = cpool.tile([P, 1], dt.float32)
    nc.vector.memset(bz[:], 0.0)

    for g in range(1, NT // 2):
        xg = xpool.tile([P, 2 * W], dt.int32, tag="x")
        nc.sync.dma_start(
            xg[:].rearrange("p (t c) -> p t c", t=2),
            label_ap[2 * P * g : 2 * P * (g + 1), :].rearrange("(t p) c -> p t c", p=P),
        )
        items.append((2 * g, xg[:, 0:W], 0))
        items.append((2 * g + 1, xg[:, W : 2 * W], 0))

    # Packed row-pooled counts: partitions 0-63 = lo plane (block-rows in
    # order), 64-127 = hi plane.  Fields are 5 bits at bits 0,5,10,15,20.
    # Planes live in separate PSUM bank pairs (cols 0:1024 / 1024:2048)
    # because accumulation-group zeroing is bank-wide across partitions.
    psum = ppool.tile([P, 2 * W], dt.float32)

    for t, x, coff in items:
        width = x.shape[-1]

        # Clampless encodings: the int16 result is the bf16 bit pattern of
        # 2^(15-5x) (lo, ACT Copy affine) / 2^(5x-10) (hi, DVE).  Out-of-
        # plane classes become fractions 2^-5..2^-20 whose per-block total
        # stays < 1; field 0 absorbs the bounded round-up.
        el = epool.tile([P, W], dt.int16, tag="el")
        nc.scalar.activation(
            el[:, 0:width], x, mybir.ActivationFunctionType.Copy,
            bias=18176.0, scale=-640.0,
        )
        eh = epool.tile([P, W], dt.int16, tag="eh")
        nc.vector.tensor_scalar(eh[:, 0:width], x, 640, 14976, aop.mult, aop.add)

        planes = ((0, el), (1, eh)) if t < NT - 1 else ((1, eh), (0, el))
        for plane, e in planes:
            base = 64 * plane
            rhs = e[:].bitcast(dt.bfloat16)
            for h in range(width // 512):
                col = coff + 512 * h
                nc.tensor.matmul(
                    psum[base : base + 64, W * plane + col : W * plane + col + 512],
                    pw[:, 64 * t : 64 * (t + 1)],
                    rhs[:, 512 * h : 512 * (h + 1)],
                    start=(t == 0),
                    stop=(t == NT - 1),
                )

    # Downstream: unpack fields, col-pool, threshold, combine.  The casts
    # run on ACT (Relu == identity for the non-negative packed sums), which
    # is idle once the encodings finish; fp32 -> int32 converts on output.
    vi = spool.tile([P, W], dt.int32, tag="vi")
    nc.scalar.activation(
        vi[0:64, :],
        psum[0:64, 0:W],
        mybir.ActivationFunctionType.Relu,
        bias=bz[0:64, :],
        scale=1.0,
    )
    nc.vector.tensor_copy(vi[64:128, :], psum[64:128, W : 2 * W])

    # Paired field extraction: fields (0,2) and (1,3) extracted together 10
    # bits apart (block sums <= 256 < 1024, so no cross-field carry), field
    # 4 alone.  Then col-pool 16, split pairs, threshold, weight.
    PAIRMASK = 31 | (31 << 10)
    fkw = fpool.tile([P, 2 * W], dt.int32, tag="fkw")
    for j, shift in enumerate((5, 10)):
        nc.vector.tensor_scalar(
            fkw[:, W * j : W * (j + 1)], vi[:], shift, PAIRMASK,
            aop.logical_shift_right, aop.bitwise_and,
        )
    rw = spool.tile([P, 2 * TW], dt.int32, tag="rw")
    # int32 accumulation of small exact ints; one reduce covers both pairs.
    with nc.allow_low_precision(reason="small int counts, exact"):
        nc.vector.tensor_reduce(
            rw[:],
            fkw[:].rearrange("p (b s) -> p b s", s=SC),
            mybir.AxisListType.X,
            aop.add,
        )
    red = {0: rw[:, 0:TW], 1: rw[:, TW : 2 * TW]}

    # Per-field block counts: pair j holds fields j+1 (low 10 bits) and
    # j+3 (high bits).
    cnts = {}
    for k in range(1, 5):
        j = (k - 1) % 2
        c = spool.tile([P, TW], dt.int32, tag=f"cnt{k}")
        if k < 3:
            nc.vector.tensor_scalar(c[:], red[j], 1023, None, aop.bitwise_and)
        else:
            nc.vector.tensor_scalar(c[:], red[j], 10, None, aop.logical_shift_right)
        cnts[k] = c

    acc = None
    for k in range(1, 5):
        mk = spool.tile([P, TW], dt.int32, tag=f"mk{k}")
        nc.vector.tensor_scalar(mk[:], cnts[k][:], 192, None, aop.is_ge)
        if acc is None:
            acc = spool.tile([P, TW], dt.bfloat16, tag=f"acc{k}")
            nc.vector.tensor_scalar(acc[:], mk[:], wc[:, k : k + 1], None, aop.mult)
        else:
            acc2 = spool.tile([P, TW], dt.bfloat16, tag=f"acc{k}")
            nc.vector.scalar_tensor_tensor(
                acc2[:], mk[:], wc[:, k : k + 1], acc[:], aop.mult, aop.add
            )
            acc = acc2

    # Cross-partition combine on the PE: fold pattern sums partitions p and
    # p+64 into psum row p (values <= 7, exact in bf16).
    psum_f = ppool.tile([TH, TW], dt.float32, tag="psum_f")
    nc.tensor.matmul(psum_f[:], pw[:, 512:576], acc[:], start=True, stop=True)
    resi = spool.tile([TH, TW], dt.int32, tag="resi")
    nc.vector.tensor_scalar(resi[:], psum_f[:], 1, None, aop.subtract)
    nc.sync.dma_start(out_ap, resi[:])


def _split_multi_waits(nc):
    """This toolchain's walrus codegen accepts at most ONE semaphore wait per
    engine instruction (two on EventSemaphore).  The Tile scheduler sometimes
    emits more; spill the extras onto same-engine NoOp carriers inserted just
    before the instruction (engines dispatch in order, so the carrier's wait
    is satisfied before the instruction issues -- semantics preserved)."""
    _ensure_path()
    from concourse import mybir

    for func in nc.m.functions:
        for blk in func.blocks:
            insts = blk.instructions
            out = []
            changed = False
            for ins in insts:
                si = ins.sync_info
                cap = 2 if isinstance(ins, mybir.InstEventSemaphore) else 1
                if si and si.on_wait and len(si.on_wait) > cap:
                    waits = list(si.on_wait)
                    for w in waits[:-cap]:
                        out.append(
                            mybir.InstNoOp(
                                name=nc.get_next_instruction_name(),
                                engine=ins.engine,
                                sync_info=mybir.SyncInfo(on_wait=[w], on_update=[]),
                                bass_nofuse=True,
                            )
                        )
                    si.on_wait = waits[-cap:]
                    changed = True
                out.append(ins)
            if changed:
                blk.instructions = out


def _install_ntff_hook():
    """Provide antenv.axon_hooks + the ctypes NTFF profile hook when the
    agent image lacks them (mirrors trn_agent_boot.trn_boot section 6)."""
    import contextlib
    import ctypes
    import types

    try:
        from antenv.axon_hooks import get_axon_ntff_profile_hook  # noqa: F401

        return
    except ImportError:
        pass
    _ensure_path()
    import antenv

    so_path = "/opt/axon/libaxon_pjrt.so"
    try:
        lib = ctypes.CDLL(so_path)
    except OSError:
        return
    if not hasattr(lib, "axon_start_nrt_profile"):
        return
    lib.axon_start_nrt_profile.argtypes = [
        ctypes.POINTER(ctypes.c_int64),
        ctypes.c_size_t,
    ]
    lib.axon_start_nrt_profile.restype = ctypes.c_int64
    lib.axon_stop_nrt_profile.argtypes = [ctypes.c_char_p]
    lib.axon_stop_nrt_profile.restype = ctypes.c_int64

    @contextlib.contextmanager
    def _hook(output_dir, device_ids):
        import jax

        jax.devices()
        if device_ids:
            ids = (ctypes.c_int64 * len(device_ids))(*device_ids)
            rc = lib.axon_start_nrt_profile(ids, len(device_ids))
        else:
            rc = lib.axon_start_nrt_profile(None, 0)
        if rc != 0:
            raise RuntimeError(f"axon_start_nrt_profile rc={rc}")
        try:
            yield
        finally:
            n = lib.axon_stop_nrt_profile(str(output_dir).encode())
            print(f"ntff profile: {n} file(s) written to {output_dir}", file=sys.stderr)

    mod = types.ModuleType("antenv.axon_hooks")
    _h = [_hook]
    mod.set_axon_ntff_profile_hook = lambda h: _h.__setitem__(0, h)
    mod.get_axon_ntff_profile_hook = lambda: _h[0]
    sys.modules["antenv.axon_hooks"] = mod
    antenv.axon_hooks = mod

    # upload_artifacts pushes the NEFF dir to a cloud bucket; keep local.
    from concourse import bass_utils as _bu

    _bu.upload_artifacts = lambda tmpdir: tmpdir


_NC_CACHE = None


def _build_nc():
    global _NC_CACHE
    if _NC_CACHE is not None:
        return _NC_CACHE
    _ensure_path()
    from contextlib import ExitStack

    import concourse.bass as bass
    import concourse.tile as tile
    from concourse import mybir

    dt = mybir.dt
    nc = bass.Bass("TRN2", target_bir_lowering=False, debug=False)
    label = nc.dram_tensor("label", [H, W], dt.int32, kind="ExternalInput").ap()
    poolw = nc.dram_tensor("poolw", [P, 576], dt.bfloat16, kind="ExternalInput").ap()
    wcol = nc.dram_tensor("wcol", [P, 8], dt.float32, kind="ExternalInput").ap()
    out = nc.dram_tensor("out", [TH, TW], dt.int32, kind="ExternalOutput").ap()
    with tile.TileContext(nc) as tc:
        with ExitStack() as ctx:
            emit_downscale(ctx, tc, out, label, poolw, wcol)
    _split_multi_waits(nc)
    _NC_CACHE = nc
    return nc


def run_on_hw(label, trace=False):
    """Run on the 8 NeuronCores; returns (out [8,1,64,64] int32, exec_time_ns)."""
    _ensure_path()
    from concourse.bass_utils import run_bass_kernel_spmd

    if trace:
        _install_ntff_hook()
    nc = _build_nc()
    poolw, wcol = make_consts()
    label = np.ascontiguousarray(label, dtype=np.int32)
    in_maps = [
        {"label": label[i], "poolw": poolw, "wcol": wcol} for i in range(N_CORES)
    ]
    r = run_bass_kernel_spmd(nc, in_maps, core_ids=list(range(N_CORES)), trace=trace)
    outs = np.stack([r.results[i]["out"] for i in range(N_CORES)])
    return outs.reshape(8, 1, TH, TW).astype(np.int32), r.exec_time_ns


def kernel(label):
    out, _ = run_on_hw(label, trace=False)
    return out
